# revision 36
# baseline (speedup 1.0000x reference)
"""Trainium2 Bass kernel for PVT-style spatial-reduction attention.

Reference computation (per batch element b, data-parallel over 8 cores):
  q   = x @ Wq                                     [4096, 256]
  xsr = LN(conv4x4s4(x.reshape(64,64,256)) + srb)  [256, 256]
  k,v = xsr @ Wkv                                  [256, 256] each
  o_h = softmax(q_h k_h^T / sqrt(32)) v_h          8 heads of 32
  y   = concat_h(o_h) @ Wp + bp                    [4096, 256]

v3 design notes (engine-balance rewrite, f32r score path):
  - Score path (x^T, Q, conv, LN, K) in f32r: softmax amplifies logit
    errors, bf16 there cost 1.8e-2 rel err on HW. Output path (e, V,
    Osc, Wp) in bf16: probability-weighted sums tolerate it.
  - conv reads patches straight out of x^T via a strided moving AP
    (no gather pass); conv output is feature-major; LN stats via
    ones-matmuls + PE broadcast (no LN transposes).
  - softmax denominator via ones-matmul col-packed like PV; recip +
    divide fused into the PSUM evacuation on DVE.
  - exp on ACT (table Exp); optional Schraudolph-on-DVE split via
    DVE_EXP for engine balance.
"""

import os
import sys

import numpy as np

sys.path.insert(0, "/opt/trn_rl_repo")
os.environ.setdefault("MYCRO_LOCAL_CACHE", "1")

B, N_TOK, DIM = 8, 4096, 256
NH, HD = 8, 32
SR = 4
GRID = 16              # 64/SR
M_KV = GRID * GRID     # 256
LN_EPS = 1e-3
SCALE = float(HD) ** -0.5
CHUNK = 512            # query-token chunk
NCH = N_TOK // CHUNK   # 8
P = 128

# Schraudolph bf16 exp: bits(e^s) ~= int16(A*s_raw + B), s_raw = raw q.k
EXP_A = 128.0 * 1.4426950408889634 * SCALE
EXP_B = 127.0 * 128.0 - 7.42 + 0.5

# how many of the 8 (mt, hp, j) exp tile slots per chunk go to DVE (approx)
DVE_EXP_N = 0
NODENOM = False
DVE_EXP = lambda mt, hp, j: (4 * mt + 2 * hp + j) >= 8 - DVE_EXP_N

LAST_RESULTS = None    # test.py introspects this for profiling info


def build_program(niter=1, loop_n=None):
    import concourse.bass as bass
    import concourse.tile as tile
    from concourse import bacc, mybir
    from concourse.masks import make_identity

    f32 = mybir.dt.float32
    f32r = mybir.dt.float32r
    bf16 = mybir.dt.bfloat16
    i16 = mybir.dt.int16
    ALU = mybir.AluOpType
    ACT = mybir.ActivationFunctionType
    AX = mybir.AxisListType

    def r(ap):
        return ap.bitcast(f32r)

    nc = bacc.Bacc("TRN2", target_bir_lowering=False, debug=False)

    x_d = nc.dram_tensor("x", (N_TOK, DIM), f32, kind="ExternalInput")
    wq_d = nc.dram_tensor("Wq", (DIM, DIM), f32, kind="ExternalInput")
    wkv_d = nc.dram_tensor("Wkv", (DIM, 2 * DIM), f32, kind="ExternalInput")
    srk_d = nc.dram_tensor("sr_kernel", (SR, SR, DIM, DIM), f32, kind="ExternalInput")
    srb_d = nc.dram_tensor("sr_bias", (DIM,), f32, kind="ExternalInput")
    gam_d = nc.dram_tensor("ln_gamma", (DIM,), f32, kind="ExternalInput")
    bet_d = nc.dram_tensor("ln_beta", (DIM,), f32, kind="ExternalInput")
    wp_d = nc.dram_tensor("Wp", (DIM, DIM), f32, kind="ExternalInput")
    bp_d = nc.dram_tensor("bp", (DIM,), f32, kind="ExternalInput")
    y_d = nc.dram_tensor("y", (N_TOK, DIM), f32, kind="ExternalOutput")

    with tile.TileContext(nc) as tc:
        import contextlib
        loop_ctx = (lambda: tc.For_i(0, loop_n, 1)) if loop_n else contextlib.nullcontext
        for _it in range(niter):
          with loop_ctx():
            with tc.tile_pool(name="persist", bufs=1) as pp:
                # ---- persistent SBUF tensors ----
                xT = pp.tile([P, 2, N_TOK], f32r)        # x^T  feature-major
                QT = pp.tile([P, 2, N_TOK], f32r)        # q^T  feature-major
                Osc = pp.tile([P, 2, N_TOK], bf16)      # (attn out)^T, normalized
                KT = pp.tile([P, 2, M_KV], f32r)         # k^T  feature-major
                Vtm = pp.tile([P, 2, DIM], bf16)        # v    token-major
                wp_b = pp.tile([P, 2, DIM], bf16)
                ones32b = pp.tile([P, 32], bf16)        # denominator stationary
                nc.gpsimd.memset(ones32b[:], 1.0)
                ones32r = pp.tile([P, 32], f32r)        # LN column-sum stationary
                ones32T = pp.tile([32, P], f32r)        # LN broadcast stationary
                # mu/rstd live replicated across 32 partitions; the broadcast
                # matmul sums all 32 copies, so scale by 1/32.
                ones_f32_scratch = pp.tile([P, 32], f32)
                nc.gpsimd.memset(ones_f32_scratch[:], 1.0)
                nc.vector.tensor_copy(ones32r[:], ones_f32_scratch[:])
                ones_f32_scr2 = pp.tile([32, P], f32)
                nc.gpsimd.memset(ones_f32_scr2[:], 1.0 / 32.0)
                nc.vector.tensor_copy(ones32T[:], ones_f32_scr2[:])
                btot_full = pp.tile([P, DIM], f32)      # broadcast bias for y
                warm_sb = pp.tile([1, 2], bf16)         # exp-table prewarm scratch
                ident = pp.tile([P, P], f32)
                make_identity(nc, ident[:])

                # ======== weight loads (overlap the x pipeline) ====
                with tc.tile_pool(name="wts", bufs=1) as wpool:
                    wq_f = wpool.tile([P, 2, DIM], f32r)
                    wq_r2 = wq_d.rearrange("(ko ki) j -> ki ko j", ki=P).bitcast(f32r)
                    for k in range(2):
                        nc.scalar.dma_start(wq_f[:, k, :], wq_r2[:, k, :])

                    wkv_f = wpool.tile([P, 2, 2 * DIM], f32r)
                    wkv_r2 = wkv_d.rearrange("(ko ki) j -> ki ko j", ki=P).bitcast(f32r)
                    for k in range(2):
                        nc.scalar.dma_start(wkv_f[:, k, :], wkv_r2[:, k, :])

                    wp_f = wpool.tile([P, 2, DIM], f32r)
                    wp_r2 = wp_d.rearrange("(ko ki) j -> ki ko j", ki=P).bitcast(f32r)
                    for k in range(2):
                        nc.scalar.dma_start(wp_f[:, k, :], wp_r2[:, k, :])
                    for k in range(2):
                        nc.vector.tensor_copy(wp_b[:, k, :], wp_f[:, k, :])

                    gam_sb = wpool.tile([P, 2], f32)
                    nc.scalar.dma_start(gam_sb[:], gam_d.rearrange("(ko ki) -> ki ko", ki=P))
                    bet_f = wpool.tile([P, 2], f32)
                    nc.scalar.dma_start(bet_f[:], bet_d.rearrange("(ko ki) -> ki ko", ki=P))
                    bet2 = wpool.tile([P, 2, 2], f32r)
                    for k in range(2):
                        for c2 in range(2):
                            nc.vector.tensor_copy(bet2[:, k, c2:c2 + 1], bet_f[:, k:k + 1])
                    srb_sb = wpool.tile([P, 2], f32)
                    nc.scalar.dma_start(srb_sb[:], srb_d.rearrange("(ko ki) -> ki ko", ki=P))
                    bp_row = wpool.tile([1, DIM], f32)
                    nc.scalar.dma_start(bp_row[:], bp_d[None, :])
                    eps_col = wpool.tile([32, 1], f32)
                    nc.gpsimd.memset(eps_col[:], LN_EPS)
                    ones_row = wpool.tile([1, P], f32r)
                    ones_row_f = wpool.tile([1, P], f32)
                    nc.gpsimd.memset(ones_row_f[:], 1.0)
                    nc.vector.tensor_copy(ones_row[:], ones_row_f[:])

                    # sr_kernel: [ci_lo 128, ci_hi 2, tap 16, co 256] f32
                    srk_f = wpool.tile([P, 2, SR * SR, DIM], f32r)
                    srk_r = srk_d.rearrange(
                        "kh kw (c2 c1) co -> c1 c2 (kh kw) co", c1=P).bitcast(f32r)
                    for k in range(2):
                        for tpair in range(0, 16, 8):
                            nc.scalar.dma_start(
                                srk_f[:, k, tpair:tpair + 8, :],
                                srk_r[:, k, tpair:tpair + 8, :])

                    # ======== prologue A: x load + PE transpose + Q proj ===
                    with (
                        tc.tile_pool(name="proA", bufs=1) as proA,
                        tc.tile_pool(name="psA", bufs=1, space="PSUM") as psA,
                    ):
                        x_sb = proA.tile([P, 32, DIM], f32)
                        x_r = x_d.rearrange("(to ti) d -> ti to d", ti=P)
                        for tt in range(0, 32, 8):
                            nc.sync.dma_start(x_sb[:, tt:tt + 8, :], x_r[:, tt:tt + 8, :])
                        # transpose x in groups of 4 -> one bank -> one evac
                        for g in range(16):
                            tp_ps = psA.tile([P, 4, P], f32, name="tp_ps", bufs=2)
                            for u in range(4):
                                blk = 4 * g + u      # 64 blocks: (tt, k)
                                tt, k = blk // 2, blk % 2
                                nc.tensor.transpose(
                                    tp_ps[:, u, :], x_sb[:, tt, k * P:(k + 1) * P],
                                    ident[:])
                            blk0 = 4 * g
                            # blocks alternate k for fixed tt: lay out as
                            # xT[:, k, tt*128:(tt+1)*128] pairs
                            for u in range(4):
                                blk = blk0 + u
                                tt, k = blk // 2, blk % 2
                                nc.vector.tensor_copy(
                                    xT[:, k, tt * P:(tt + 1) * P], tp_ps[:, u, :])

                        # beta contribution: K-bias is softmax-invariant; V-bias
                        # flows through Wp into a per-output-dim constant.
                        bvT = proA.tile([P, 2, 2], f32r)
                        for ko in range(2):
                            bv_ps = psA.tile([P, 2], f32, name="bv_ps", bufs=1)
                            for k in range(2):
                                nc.tensor.matmul(
                                    bv_ps[:],
                                    r(wkv_f[:, k, DIM + ko * P:DIM + (ko + 1) * P]),
                                    r(bet2[:, k, :]),
                                    start=(k == 0), stop=(k == 1),
                                )
                            nc.vector.tensor_copy(bvT[:, ko, :], bv_ps[:])
                        bt_ps = psA.tile([1, DIM], f32, name="bt_ps", bufs=1)
                        for k in range(2):
                            nc.tensor.matmul(
                                bt_ps[:], r(bvT[:, k, 0:1]), r(wp_f[:, k, :]),
                                start=(k == 0), stop=(k == 1),
                            )
                        btot_row = proA.tile([1, DIM], f32r)
                        nc.vector.tensor_add(btot_row[:], bt_ps[:], bp_row[:])
                        btot_bc_ps = psA.tile([P, DIM], f32, name="btot_bc", bufs=1)
                        nc.tensor.matmul(btot_bc_ps[:], r(ones_row[:]), r(btot_row[:]),
                                         start=True, stop=True)
                        nc.vector.tensor_copy(btot_full[:], btot_bc_ps[:])

                        for ko in range(2):
                            for c in range(NCH):
                                qt_ps = psA.tile([P, CHUNK], f32, name="qt_ps", bufs=2)
                                for k in range(2):
                                    nc.tensor.matmul(
                                        qt_ps[:],
                                        r(wq_f[:, k, ko * P:(ko + 1) * P]),
                                        r(xT[:, k, c * CHUNK:(c + 1) * CHUNK]),
                                        start=(k == 0), stop=(k == 1),
                                    )
                                nc.scalar.copy(
                                    QT[:, ko, c * CHUNK:(c + 1) * CHUNK], qt_ps[:])

                    # ======== prologue B: conv + LN + K/V (baseline structure) ====
                    with (
                        tc.tile_pool(name="proB", bufs=1) as proB,
                        tc.tile_pool(name="psB", bufs=1, space="PSUM") as psB,
                    ):
                        # gamma folded into Wkv rows (per input-dim partition)
                        wkv_r = proB.tile([P, 2, 2 * DIM], f32r)
                        for k in range(2):
                            nc.vector.tensor_scalar_mul(
                                wkv_r[:, k, :], wkv_f[:, k, :], gam_sb[:, k:k + 1])
                        srb_row = proB.tile([1, DIM], f32r)
                        nc.scalar.dma_start(srb_row[:], srb_d[None, :].bitcast(f32r))
                        srb_full = proB.tile([P, DIM], f32)
                        srb_bc_ps = psB.tile([P, DIM], f32, name="srb_bc", bufs=1)
                        nc.tensor.matmul(srb_bc_ps[:], r(ones_row[:]), r(srb_row[:]),
                                         start=True, stop=True)
                        nc.vector.tensor_copy(srb_full[:], srb_bc_ps[:])
                        eps_colP = proB.tile([P, 1], f32)
                        nc.gpsimd.memset(eps_colP[:], LN_EPS)

                        # gather strided conv patches into contiguous tiles
                        xT_p = xT.rearrange(
                            "p k (i di j dj) -> p k i di j dj", di=SR, dj=SR, j=GRID)
                        xTp = proB.tile([P, 2, SR * SR, M_KV], f32r)
                        for di in range(SR):
                            for dj in range(SR):
                                for k in range(2):
                                    nc.vector.tensor_copy(
                                        xTp[:, k, SR * di + dj, :],
                                        xT_p[:, k, :, di, :, dj],
                                    )

                        xlnT = proB.tile([P, 2, M_KV], f32r)
                        for mt in range(2):
                            conv_ps = psB.tile([P, DIM], f32, name="conv_ps", bufs=1)
                            idx = 0
                            for tap in range(SR * SR):
                                for k in range(2):
                                    nc.tensor.matmul(
                                        conv_ps[:],
                                        r(xTp[:, k, tap, mt * P:(mt + 1) * P]),
                                        r(srk_f[:, k, tap, :]),
                                        start=(idx == 0), stop=(idx == 31),
                                    )
                                    idx += 1
                            tmp = proB.tile([P, DIM], f32, name="ln_tmp", bufs=2)
                            nc.vector.tensor_add(tmp[:], conv_ps[:], srb_full[:])
                            musum = proB.tile([P, 1], f32, name="ln_mu", bufs=2)
                            nc.vector.tensor_reduce(musum[:], tmp[:], axis=AX.X, op=ALU.add)
                            xc = proB.tile([P, DIM], f32, name="ln_xc", bufs=2)
                            nc.vector.scalar_tensor_tensor(
                                xc[:], musum.to_broadcast([P, DIM]), -1.0 / DIM, tmp[:],
                                op0=ALU.mult, op1=ALU.add,
                            )
                            sq = proB.tile([P, DIM], f32, name="ln_sq", bufs=2)
                            varsum = proB.tile([P, 1], f32, name="ln_var", bufs=2)
                            nc.scalar.activation(sq[:], xc[:], ACT.Square, accum_out=varsum[:])
                            sd = proB.tile([P, 1], f32, name="ln_sd", bufs=2)
                            nc.scalar.activation(sd[:], varsum[:], ACT.Sqrt,
                                                 bias=eps_colP[:, :], scale=1.0 / DIM)
                            rstd = proB.tile([P, 1], f32, name="ln_rstd", bufs=2)
                            nc.vector.reciprocal(rstd[:], sd[:])
                            xln = proB.tile([P, DIM], f32, name="ln_out", bufs=2)
                            nc.vector.tensor_scalar_mul(xln[:], xc[:], rstd[:])
                            for k in range(2):
                                t_ps = psB.tile([P, P], f32, name="t_ps", bufs=1)
                                nc.tensor.transpose(t_ps[:], xln[:, k * P:(k + 1) * P], ident[:])
                                nc.vector.tensor_copy(xlnT[:, k, mt * P:(mt + 1) * P], t_ps[:])

                        # K^T feature-major (f32)
                        for ko in range(2):
                            kt_ps = psB.tile([P, M_KV], f32, name="kt_ps", bufs=1)
                            for k in range(2):
                                nc.tensor.matmul(
                                    kt_ps[:],
                                    r(wkv_r[:, k, ko * P:(ko + 1) * P]),
                                    r(xlnT[:, k, :]),
                                    start=(k == 0), stop=(k == 1),
                                )
                            nc.vector.tensor_copy(KT[:, ko, :], kt_ps[:])
                        # V token-major (bf16)
                        for mt in range(2):
                            v_ps = psB.tile([P, DIM], f32, name="v_ps", bufs=1)
                            for k in range(2):
                                nc.tensor.matmul(
                                    v_ps[:],
                                    r(xlnT[:, k, mt * P:(mt + 1) * P]),
                                    r(wkv_r[:, k, DIM:2 * DIM]),
                                    start=(k == 0), stop=(k == 1),
                                )
                            nc.vector.tensor_copy(Vtm[:, mt, :], v_ps[:])

                # pre-warm the Exp ACT table during prologue slack (the sqrt
                # set from LN would otherwise force the load onto the first
                # attention exp's critical path)
                nc.scalar.activation(warm_sb[:], btot_full[0:1, 0:2], ACT.Exp)

                # ======== attention + y-proj, chunked over queries ====
                with (
                    tc.tile_pool(name="attn_sb", bufs=1) as asb,
                    tc.tile_pool(name="psS", bufs=1, space="PSUM") as psS,
                    tc.tile_pool(name="psO", bufs=1, space="PSUM") as psO,
                    tc.tile_pool(name="psD", bufs=1, space="PSUM") as psD,
                ):
                    y_r = y_d.rearrange("(to ti) d -> ti to d", ti=P)
                    for c in range(NCH):
                        exps = {}
                        for hp in range(2):
                            for mt in range(2):
                                for j in range(2):
                                    # 2 heads per PSUM tile; single ring name
                                    # with bufs=2 so scores(g+1) overlaps exp(g)
                                    spt = psS.tile([P, 2 * CHUNK], f32, name="sp", bufs=3)
                                    for hi in range(2):
                                        hh = 2 * j + hi
                                        nc.tensor.matmul(
                                            spt[:, CHUNK * hi:CHUNK * (hi + 1)],
                                            r(KT[32 * hh:32 * hh + 32, hp, mt * P:(mt + 1) * P]),
                                            r(QT[32 * hh:32 * hh + 32, hp, c * CHUNK:(c + 1) * CHUNK]),
                                            start=True, stop=True,
                                            tile_position=(32 * hh, 0),
                                        )
                                    e = asb.tile([P, 2 * CHUNK], bf16, name="expS", bufs=10)
                                    if DVE_EXP(mt, hp, j):
                                        nc.vector.tensor_scalar(
                                            e[:].bitcast(i16), spt[:],
                                            EXP_A, EXP_B, op0=ALU.mult, op1=ALU.add)
                                    else:
                                        nc.scalar.activation(
                                            e[:], spt[:], ACT.Exp, scale=SCALE)
                                    exps[(mt, hp, j)] = e

                            o_ps = psO.tile([P, CHUNK], f32, name="o_ps", bufs=1)
                            d_ps = psD.tile([P, CHUNK], f32, name="d_ps", bufs=1)
                            for hh in range(4):
                                h = 4 * hp + hh
                                for mt in range(2):
                                    e_ap = exps[(mt, hp, hh // 2)][:, CHUNK * (hh % 2):CHUNK * (hh % 2) + CHUNK]
                                    nc.tensor.matmul(
                                        o_ps[32 * hh:32 * hh + 32, :],
                                        Vtm[:, mt, 32 * h:32 * h + 32],
                                        e_ap,
                                        start=(mt == 0), stop=(mt == 1),
                                        tile_position=(0, 32 * hh),
                                    )
                                if not NODENOM:
                                    for mt in range(2):
                                        e_ap = exps[(mt, hp, hh // 2)][:, CHUNK * (hh % 2):CHUNK * (hh % 2) + CHUNK]
                                        nc.tensor.matmul(
                                            d_ps[32 * hh:32 * hh + 32, :],
                                            ones32b[:],
                                            e_ap,
                                            start=(mt == 0), stop=(mt == 1),
                                            tile_position=(0, 32 * hh),
                                        )
                            if NODENOM:
                                nc.vector.tensor_copy(
                                    Osc[:, hp, c * CHUNK:(c + 1) * CHUNK], o_ps[:])
                            else:
                                dr = asb.tile([P, CHUNK], f32, name="dr", bufs=4)
                                nc.vector.reciprocal_approx_fast(dr[:], d_ps[:])
                                nc.vector.tensor_mul(
                                    Osc[:, hp, c * CHUNK:(c + 1) * CHUNK], o_ps[:], dr[:]
                                )

                        # y projection for this chunk (reuses psO ring banks)
                        y_sb = asb.tile([P, 4, DIM], f32, name="y_sb", bufs=2)
                        for half in range(2):
                            y_ps = psO.tile([P, CHUNK], f32, name="o_ps", bufs=1)
                            for tl in range(2):
                                tt = 4 * c + 2 * half + tl
                                for k in range(2):
                                    nc.tensor.matmul(
                                        y_ps[:, tl * DIM:(tl + 1) * DIM],
                                        Osc[:, k, tt * P:(tt + 1) * P],
                                        wp_b[:, k, :],
                                        start=(k == 0), stop=(k == 1),
                                    )
                            for tl in range(2):
                                nc.vector.scalar_tensor_tensor(
                                    y_sb[:, 2 * half + tl, :],
                                    y_ps[:, tl * DIM:(tl + 1) * DIM], 0.0,
                                    btot_full[:],
                                    op0=ALU.bypass, op1=ALU.add,
                                )
                        nc.sync.dma_start(y_r[:, 4 * c:4 * c + 4, :], y_sb[:, :, :])

    return nc


def kernel(**inputs):
    global LAST_RESULTS
    from concourse.bass_utils import run_bass_kernel_spmd

    f = lambda a: np.ascontiguousarray(np.asarray(a, dtype=np.float32))
    x = f(inputs["x"])
    shared = {
        k: f(inputs[k])
        for k in ("Wq", "Wkv", "sr_kernel", "sr_bias", "ln_gamma", "ln_beta", "Wp", "bp")
    }
    nc = build_program()
    if not nc.is_finalized():
        nc.finalize()
    in_maps = [dict(x=x[b], **shared) for b in range(B)]
    res = run_bass_kernel_spmd(
        nc, in_maps, core_ids=list(range(B)),
        trace=bool(int(os.environ.get("KERNEL_TRACE", "0"))),
    )
    LAST_RESULTS = res
    return np.stack([r["y"] for r in res.results], axis=0)
